# revision 11
# baseline (speedup 1.0000x reference)
"""SPGAT (single-layer GAT, batch=1) Trainium2 kernel, 8-core row-parallel.

Math (reference):
    Wh  = inputs @ W                          [N, D]
    f1  = Wh @ a1, f2 = Wh @ a2               [N, 1]
    e   = leaky_relu(f1 + f2.T, 0.2)          [N, N]
    att = softmax(where(adj > 0, e, -inf))    [N, N]
    out = relu(att @ Wh)                      [N, D]

Key reformulations:
  * Masked softmax == multiply exp(e) by the 0/1 adjacency and normalize by
    the masked row-sum (exact; adj is 0/1).  Normalization is deferred past
    the aggregation matmul: out_r = relu((P @ Wh)_r / s_r) with
    P = adj * exp(e); s_r comes free from a ones-column appended to Wh.
  * exp is monotone and each softmax row is scale-invariant; dividing row r
    by exp(f1[r]) gives
        t0[c, r] = max(b1[c], g[r] * b2[c]),
        g = exp(-0.8 f1), b1 = exp(f2), b2 = exp(0.2 f2),
    so no dense transcendentals remain.  Per [128, R] chunk the work is
    either (DVE form) one dual-scalar tensor_scalar (4x mode) plus one
    tensor_tensor mask multiply, or (ScalarE form, used on 5 of 8 chunks
    for engine balance) tr = Relu(b2*g - b1) on ScalarE followed by ONE
    fused (tr + b1) * adj scalar_tensor_tensor on VectorE.
  * Adjacency streams through BOTH DGE rings concurrently: half the
    c-chunks as bf16 over HWDGE (sync) and half as fp8e4m3 (0/1 exact)
    over SWDGE (gpsimd) with on-the-fly cast to bf16 -- halving that
    half's HBM reads and running the two queues in parallel, in 2 MB
    octo-chunk transfers that amortize per-op fixed costs.
  * Everything N x N is produced directly in transposed [c, r] layout so the
    PE contraction (over c) needs no on-device transposes: per c-chunk the
    8 lhsT slices feed 8 PSUM accumulators [128, D+1] (one per row block).
  * A burst of dummy warm-up matmuls during the initial DMA fill flips the
    PE HAM clock-gate to 8/8 before the real matmul stream begins.

Sharding: rows split 1024/core over 8 cores; per-core adj^T column block is
host-prepared.  The O(N D^2) projections (Wh and the rank-1 f1/f2/exp
vectors, ~3% of FLOPs) are host prep, replicated to all cores; all O(N^2)
attention work (34 GFLOP) runs on-device.  No collectives are needed.
"""

import os
import sys

import numpy as np

try:
    import concourse.bass as bass  # noqa: F401
except Exception:  # pragma: no cover - grading env fallback
    for p in ("/opt/trn_rl_repo", "/root/.axon_site/_ro/trn_rl_repo"):
        if os.path.isdir(p) and p not in sys.path:
            sys.path.insert(0, p)
    import concourse.bass as bass  # noqa: F401

import ml_dtypes

import concourse.tile as tile
from concourse import bacc, bass_utils, mybir

N = 8192
D = 256
NCORES = 8
R = N // NCORES   # rows per core = 1024
RT = R // 128     # r tiles per core = 8
CT = N // 128     # c tiles = 64
NO = CT // 16     # octo-groups of 16 chunks = 4 (8 SW + 8 HW each)
ALPHA = 0.2

F32 = mybir.dt.float32
BF16 = mybir.dt.bfloat16
FP8E4 = mybir.dt.float8e4
BF16_NP = ml_dtypes.bfloat16
FP8E4_NP = ml_dtypes.float8_e4m3fn

AF = mybir.ActivationFunctionType
OP = mybir.AluOpType


def act_form(t):
    # chunks handled by the ScalarE Relu form (5 of 8; rest DVE form)
    return t % 8 < 5


def build_nc():
    nc = bacc.Bacc("TRN2", target_bir_lowering=False, debug=False,
                   num_devices=NCORES)

    # octo layouts: row o*128+p holds 8 c-chunks side by side
    # SW octo o covers chunks 16o..16o+7 (fp8, cast to bf16 in the DMA);
    # HW octo o covers chunks 16o+8..16o+15 (bf16 direct).
    adj8_d = nc.dram_tensor("adj8", [NO * 128, 8 * R], FP8E4,
                            kind="ExternalInput")
    adjb_d = nc.dram_tensor("adjb", [NO * 128, 8 * R], BF16,
                            kind="ExternalInput")
    whp_d = nc.dram_tensor("whp", [128, CT * (D + 1)], BF16,
                           kind="ExternalInput")
    gbp_d = nc.dram_tensor("gbp", [128, R], BF16, kind="ExternalInput")
    bv_d = nc.dram_tensor("bv", [128, 3, CT], F32, kind="ExternalInput")
    out_d = nc.dram_tensor("out", [R, D], F32, kind="ExternalOutput")

    with tile.TileContext(nc) as tc:
        with (
            tc.tile_pool(name="const", bufs=1) as cpool,
            tc.tile_pool(name="sw", bufs=3) as swp,
            tc.tile_pool(name="hw", bufs=3) as hwp,
            tc.tile_pool(name="work", bufs=6) as work,
            tc.tile_pool(name="pt", bufs=8) as pt,
            tc.tile_pool(name="fin", bufs=4) as fin,
            tc.tile_pool(name="rp", bufs=8) as rp,
            tc.tile_pool(name="ps", bufs=8, space=bass.MemorySpace.PSUM) as ps,
        ):
            # ---------------- constants ----------------
            gbp = cpool.tile([128, R], BF16, name="gbp")  # exp(-0.8 f1[r])
            nc.sync.dma_start(gbp[:], gbp_d[:, :])        # host pre-broadcast
            bv = cpool.tile([128, 3, CT], F32, name="bv")  # b2 | b1 | -b1
            nc.sync.dma_start(bv[:], bv_d[:, :, :])
            b2c = bv[:, 0, :]
            b1c = bv[:, 1, :]
            nb1c = bv[:, 2, :]

            whp = cpool.tile([128, CT, D + 1], BF16, name="whp")

            # ------- accumulators (live across the c loop) -------
            accs = [ps.tile([128, D + 1], F32, tag="ps", name=f"acc{j}")
                    for j in range(RT)]

            # HAM warm-up: dummy matmuls on the (early, tiny) gbp tile keep
            # the PE busy through the initial DMA fill so the clock gate is
            # at 8/8 when the real stream starts.  accs[6]/accs[7] are used
            # as scratch; the real t=0 matmul has start=True and overwrites.
            for w in range(14):
                nc.tensor.matmul(accs[6 + (w % 2)][:, :], gbp[:, 0:128],
                                 gbp[:, 0:257], start=True, stop=True)

            # adjacency + whp streams (SW ring starts the first chunks so
            # the PE can begin while the sync ring fills whp)
            sw_tiles, hw_tiles = [], []
            for o in range(NO):
                swt = swp.tile([128, 8, R], BF16, tag="sw", name=f"sw{o}")
                nc.gpsimd.dma_start(swt[:, :, :],
                                    adj8_d[o * 128:(o + 1) * 128, :])
                sw_tiles.append(swt)
            for o in range(NO):
                nc.sync.dma_start(
                    whp[:, 16 * o:16 * (o + 1), :],
                    whp_d[:, 16 * o * (D + 1):16 * (o + 1) * (D + 1)])
                hwt = hwp.tile([128, 8, R], BF16, tag="hw", name=f"hw{o}")
                nc.sync.dma_start(hwt[:, :, :],
                                  adjb_d[o * 128:(o + 1) * 128, :])
                hw_tiles.append(hwt)

            # ------------- main loop over c chunks -------------
            for t in range(CT):
                o, e = t // 16, t % 16
                adj = (sw_tiles[o][:, e, :] if e < 8
                       else hw_tiles[o][:, e - 8, :])
                p = pt.tile([128, R], BF16, tag="p", name=f"p{t}")
                if act_form(t):
                    # tr = relu(b2*g - b1); p = (tr + b1) * adj
                    tr = work.tile([128, R], BF16, tag="tr", name=f"tr{t}")
                    nc.scalar.activation(tr[:], gbp[:], AF.Relu,
                                         bias=nb1c[:, t:t + 1],
                                         scale=b2c[:, t:t + 1])
                    nc.vector.scalar_tensor_tensor(p[:], tr[:],
                                                   b1c[:, t:t + 1], adj,
                                                   OP.add, OP.mult)
                else:
                    # t0 = max(b2*g, b1); p = t0 * adj
                    t0 = work.tile([128, R], BF16, tag="tr", name=f"t0{t}")
                    nc.vector.tensor_scalar(t0[:], gbp[:], b2c[:, t:t + 1],
                                            b1c[:, t:t + 1], OP.mult, OP.max)
                    nc.vector.tensor_mul(p[:], t0[:], adj)
                for j in range(RT):
                    nc.tensor.matmul(
                        accs[j][:, :],
                        p[:, j * 128:(j + 1) * 128],
                        whp[:, t, :],
                        start=(t == 0), stop=(t == CT - 1),
                    )

            # ---------------- normalize + relu + store ----------------
            for j in range(RT):
                rec = rp.tile([128, 1], F32, tag="rec", name=f"rec{j}")
                nc.vector.reciprocal(rec[:], accs[j][:, D:D + 1])
                o_t = fin.tile([128, D], F32, tag="o", name=f"o{j}")
                if j % 2 == 0:
                    # relu(acc * rec) via DVE dual-op tensor_scalar
                    nc.vector.tensor_scalar(o_t[:], accs[j][:, 0:D],
                                            rec[:], 0.0, OP.mult, OP.max)
                else:
                    nc.scalar.activation(o_t[:], accs[j][:, 0:D],
                                         AF.Relu, bias=0.0, scale=rec[:])
                nc.sync.dma_start(out_d[j * 128:(j + 1) * 128, :], o_t[:])

    nc.compile()
    return nc


_CACHE = {}


def _get_nc():
    if "nc" not in _CACHE:
        _CACHE["nc"] = build_nc()
    return _CACHE["nc"]


def make_in_maps(inputs, adj, W, a1, a2):
    inputs = np.asarray(inputs, dtype=np.float32)
    adj = np.asarray(adj, dtype=np.float32)
    W = np.asarray(W, dtype=np.float32)
    a1 = np.asarray(a1, dtype=np.float32)
    a2 = np.asarray(a2, dtype=np.float32)

    # projections (~3% of FLOPs) on host, replicated to all cores
    Wh = inputs @ W
    f1 = (Wh @ a1).reshape(N).astype(np.float32)
    f2 = (Wh @ a2).reshape(N).astype(np.float32)
    whp = np.concatenate(
        [Wh, np.ones((N, 1), np.float32)], axis=1).astype(BF16_NP)
    # [128, CT*(D+1)]: row p holds [t, d] for c = t*128 + p
    whp_p = np.ascontiguousarray(
        whp.reshape(CT, 128, D + 1).transpose(1, 0, 2).reshape(128, -1))

    gp = np.exp(-(1.0 - ALPHA) * f1)          # per-row factor
    b1 = np.exp(f2)
    b2 = np.exp(ALPHA * f2)
    b1c = np.ascontiguousarray(b1.reshape(CT, 128).T)
    b2c = np.ascontiguousarray(b2.reshape(CT, 128).T)
    bv = np.ascontiguousarray(np.stack([b2c, b1c, -b1c], axis=1)
                              ).astype(np.float32)  # [128, 3, CT]

    in_maps = []
    for k in range(NCORES):
        r0, r1 = k * R, (k + 1) * R
        adjT = (adj[r0:r1, :].T > 0).astype(np.float32)  # [N, R] 0/1
        # split chunks: SW octo o = chunks 16o..16o+7, HW = 16o+8..16o+15
        a4 = adjT.reshape(NO, 16, 128, R)
        sw = np.ascontiguousarray(
            a4[:, :8].transpose(0, 2, 1, 3).reshape(NO * 128, 8 * R))
        hw = np.ascontiguousarray(
            a4[:, 8:].transpose(0, 2, 1, 3).reshape(NO * 128, 8 * R))
        in_maps.append({
            "adj8": sw.astype(FP8E4_NP),
            "adjb": hw.astype(BF16_NP),
            "whp": whp_p,
            "gbp": np.ascontiguousarray(np.broadcast_to(
                gp[r0:r1].reshape(1, R).astype(BF16_NP), (128, R))),
            "bv": bv,
        })
    return in_maps


def run(in_maps, trace=False):
    nc = _get_nc()
    res = bass_utils.run_bass_kernel_spmd(
        nc, [dict(m) for m in in_maps], core_ids=list(range(NCORES)),
        trace=trace,
    )
    out = np.concatenate([res.results[k]["out"] for k in range(NCORES)],
                         axis=0)
    return out, res


def kernel(inputs, adj, cmt_weight, W, a1, a2):
    in_maps = make_in_maps(inputs, adj, W, a1, a2)
    out, _ = run(in_maps, trace=False)
    return out.astype(np.float32)


# revision 15
# speedup vs baseline: 1.2762x; 1.2762x over previous
"""SPGAT (single-layer GAT, batch=1) Trainium2 kernel, 8-core row-parallel.

Math (reference):
    Wh  = inputs @ W                          [N, D]
    f1  = Wh @ a1, f2 = Wh @ a2               [N, 1]
    e   = leaky_relu(f1 + f2.T, 0.2)          [N, N]
    att = softmax(where(adj > 0, e, -inf))    [N, N]
    out = relu(att @ Wh)                      [N, D]

Key reformulations:
  * Masked softmax == multiply exp(e) by the 0/1 adjacency and normalize by
    the masked row-sum (exact; adj is 0/1).  Normalization is deferred past
    the aggregation matmul: out_r = relu((P @ Wh)_r / s_r) with
    P = adj * exp(e); s_r comes free from a ones-column appended to Wh.
  * exp is monotone and each softmax row is scale-invariant; dividing row r
    by exp(f1[r]) gives
        t0[c, r] = max(b1[c], g[r] * b2[c]),
        g = exp(-0.8 f1), b1 = exp(f2), b2 = exp(0.2 f2),
    so no dense transcendentals remain.  Per [128, R] chunk the work is
    either (DVE form) one dual-scalar tensor_scalar (4x mode) plus one
    tensor_tensor mask multiply, or (ScalarE form, used on 5 of 8 chunks
    for engine balance) tr = Relu(b2*g - b1) on ScalarE followed by ONE
    fused (tr + b1) * adj scalar_tensor_tensor on VectorE.
  * Adjacency streams through BOTH DGE rings concurrently: half the
    c-chunks as bf16 over HWDGE (sync) and half as fp8e4m3 (0/1 exact)
    over SWDGE (gpsimd) with on-the-fly cast to bf16 -- halving that
    half's HBM reads and running the two queues in parallel, in 2 MB
    octo-chunk transfers that amortize per-op fixed costs.
  * Everything N x N is produced directly in transposed [c, r] layout so the
    PE contraction (over c) needs no on-device transposes: per c-chunk the
    8 lhsT slices feed 8 PSUM accumulators [128, D+1] (one per row block).
  * A burst of dummy warm-up matmuls during the initial DMA fill flips the
    PE HAM clock-gate to 8/8 before the real matmul stream begins.

Sharding: rows split 1024/core over 8 cores; per-core adj^T column block is
host-prepared.  The O(N D^2) projections (Wh and the rank-1 f1/f2/exp
vectors, ~3% of FLOPs) are host prep, replicated to all cores; all O(N^2)
attention work (34 GFLOP) runs on-device.  No collectives are needed.
"""

import os
import sys

import numpy as np

try:
    import concourse.bass as bass  # noqa: F401
except Exception:  # pragma: no cover - grading env fallback
    for p in ("/opt/trn_rl_repo", "/root/.axon_site/_ro/trn_rl_repo"):
        if os.path.isdir(p) and p not in sys.path:
            sys.path.insert(0, p)
    import concourse.bass as bass  # noqa: F401

import ml_dtypes

import concourse.tile as tile
from concourse import bacc, bass_utils, mybir

N = 8192
D = 256
NCORES = 8
R = N // NCORES   # rows per core = 1024
RT = R // 128     # r tiles per core = 8
CT = N // 128     # c tiles = 64
NO = CT // 16     # octo-groups of 16 chunks = 4 (8 SW + 8 HW each)
ALPHA = 0.2

F32 = mybir.dt.float32
BF16 = mybir.dt.bfloat16
FP8E4 = mybir.dt.float8e4
BF16_NP = ml_dtypes.bfloat16
FP8E4_NP = ml_dtypes.float8_e4m3fn

AF = mybir.ActivationFunctionType
OP = mybir.AluOpType


def act_form(t):
    # chunks whose t0 is built by ScalarE (2-pass Relu+Identity) instead of
    # one DVE tensor_scalar: 5 of every 16 -> 20 chunks, sized so ScalarE
    # (~2.35 us/chunk) stays under the PE's ~70 us stream.
    return t % 16 in (0, 1, 2, 8, 9)


def build_nc():
    nc = bacc.Bacc("TRN2", target_bir_lowering=False, debug=False,
                   num_devices=NCORES)

    # octo layouts: row o*128+p holds 8 c-chunks side by side
    # SW octo o covers chunks 16o..16o+7 (fp8, cast to bf16 in the DMA);
    # HW octo o covers chunks 16o+8..16o+15 (bf16 direct).
    adj8_d = nc.dram_tensor("adj8", [NO * 128, 8 * R], FP8E4,
                            kind="ExternalInput")
    adjb_d = nc.dram_tensor("adjb", [NO * 128, 8 * R], BF16,
                            kind="ExternalInput")
    whp_d = nc.dram_tensor("whp", [128, CT * (D + 1)], BF16,
                           kind="ExternalInput")
    gbp_d = nc.dram_tensor("gbp", [128, R], BF16, kind="ExternalInput")
    bv_d = nc.dram_tensor("bv", [128, 3, CT], F32, kind="ExternalInput")
    out_d = nc.dram_tensor("out", [R, D], F32, kind="ExternalOutput")

    with tile.TileContext(nc) as tc:
        with (
            tc.tile_pool(name="const", bufs=1) as cpool,
            tc.tile_pool(name="sw", bufs=2) as swp,
            tc.tile_pool(name="hw", bufs=2) as hwp,
            tc.tile_pool(name="work", bufs=3) as work,
            tc.tile_pool(name="pt", bufs=4) as pt,
            tc.tile_pool(name="fin", bufs=4) as fin,
            tc.tile_pool(name="rp", bufs=8) as rp,
            tc.tile_pool(name="ps", bufs=8, space=bass.MemorySpace.PSUM) as ps,
        ):
            # ---------------- constants ----------------
            gbp = cpool.tile([128, R], BF16, name="gbp")  # exp(-0.8 f1[r])
            nc.sync.dma_start(gbp[:], gbp_d[:, :])        # host pre-broadcast
            bv = cpool.tile([128, 3, CT], F32, name="bv")  # b2 | b1 | -b1
            nc.sync.dma_start(bv[:], bv_d[:, :, :])
            b2c = bv[:, 0, :]
            b1c = bv[:, 1, :]
            nb1c = bv[:, 2, :]

            whp = cpool.tile([128, CT, D + 1], BF16, name="whp")

            # ------- accumulators (live across the c loop) -------
            accs = [ps.tile([128, D + 1], F32, tag="ps", name=f"acc{j}")
                    for j in range(RT)]

            # adjacency + whp streams (SW ring starts the first chunks so
            # the PE can begin while the sync ring fills whp)
            sw_tiles, hw_tiles = [], []
            for o in range(NO):
                swt = swp.tile([128, 8, R], BF16, tag="sw", name=f"sw{o}")
                nc.gpsimd.dma_start(swt[:, :, :],
                                    adj8_d[o * 128:(o + 1) * 128, :])
                sw_tiles.append(swt)
            for o in range(NO):
                nc.sync.dma_start(
                    whp[:, 16 * o:16 * (o + 1), :],
                    whp_d[:, 16 * o * (D + 1):16 * (o + 1) * (D + 1)])
                hwt = hwp.tile([128, 8, R], BF16, tag="hw", name=f"hw{o}")
                nc.sync.dma_start(hwt[:, :, :],
                                  adjb_d[o * 128:(o + 1) * 128, :])
                hw_tiles.append(hwt)

            # HAM warm-up: dummy matmuls on the (early, tiny) gbp tile keep
            # the PE busy through the initial DMA fill so the clock gate is
            # at 8/8 when the real stream starts.  accs[6]/accs[7] are used
            # as scratch; the real t=0 matmul has start=True and overwrites.
            for w in range(20):
                nc.tensor.matmul(accs[6 + (w % 2)][:, :], gbp[:, 0:128],
                                 gbp[:, 0:257], start=True, stop=True)

            # ------------- main loop over c chunks -------------
            # t0 per chunk (DVE tensor_scalar or ScalarE 2-pass) into an
            # octo-wide tile; mask multiplies batched as quad tensor_tensor
            # (per-chunk on the first octo for a fast pipeline start).
            t0_tiles = {}
            for t in range(CT):
                o, e = t // 16, t % 16
                half = e // 8              # 0 = SW tile, 1 = HW tile
                oct_id = 2 * o + half
                ee = e % 8
                if ee == 0:
                    t0_tiles[oct_id] = work.tile([128, 8, R], BF16, tag="t0",
                                                 name=f"t0o{oct_id}")
                t0 = t0_tiles[oct_id]
                if act_form(t):
                    # t0 = relu(b2*g - b1) + b1  (both passes on ScalarE)
                    tr = pt.tile([128, R], BF16, tag="tr", name=f"tr{t}")
                    nc.scalar.activation(tr[:], gbp[:], AF.Relu,
                                         bias=nb1c[:, t:t + 1],
                                         scale=b2c[:, t:t + 1])
                    nc.scalar.activation(t0[:, ee, :], tr[:], AF.Identity,
                                         bias=b1c[:, t:t + 1], scale=1.0)
                else:
                    # t0 = max(b2*g, b1) in one dual-scalar tensor_scalar
                    nc.vector.tensor_scalar(t0[:, ee, :], gbp[:],
                                            b2c[:, t:t + 1], b1c[:, t:t + 1],
                                            OP.mult, OP.max)
                adj_t = sw_tiles[o] if half == 0 else hw_tiles[o]
                if o == 0:
                    # chunk-granular masks for the first 16 chunks
                    p = pt.tile([128, R], BF16, tag="p", name=f"p{t}")
                    nc.vector.tensor_mul(p[:], t0[:, ee, :], adj_t[:, ee, :])
                    mm_srcs = [(t, p[:, :])]
                elif ee % 4 == 3:
                    # one quad tensor_tensor covers chunks t-3..t
                    pq = pt.tile([128, 4, R], BF16, tag="pq", bufs=3,
                                 name=f"pq{t}")
                    q0 = ee - 3
                    nc.vector.tensor_mul(pq[:, :, :], t0[:, q0:q0 + 4, :],
                                         adj_t[:, q0:q0 + 4, :])
                    mm_srcs = [(t - 3 + i, pq[:, i, :]) for i in range(4)]
                else:
                    mm_srcs = []
                for tt, psrc in mm_srcs:
                    for j in range(RT):
                        nc.tensor.matmul(
                            accs[j][:, :],
                            psrc[:, j * 128:(j + 1) * 128],
                            whp[:, tt, :],
                            start=(tt == 0), stop=(tt == CT - 1),
                        )

            # ---------------- normalize + relu + store ----------------
            for j in range(RT):
                rec = rp.tile([128, 1], F32, tag="rec", name=f"rec{j}")
                nc.vector.reciprocal(rec[:], accs[j][:, D:D + 1])
                o_t = fin.tile([128, D], F32, tag="o", name=f"o{j}")
                if j % 2 == 0:
                    # relu(acc * rec) via DVE dual-op tensor_scalar
                    nc.vector.tensor_scalar(o_t[:], accs[j][:, 0:D],
                                            rec[:], 0.0, OP.mult, OP.max)
                else:
                    nc.scalar.activation(o_t[:], accs[j][:, 0:D],
                                         AF.Relu, bias=0.0, scale=rec[:])
                nc.sync.dma_start(out_d[j * 128:(j + 1) * 128, :], o_t[:])

    nc.compile()
    return nc


_CACHE = {}


def _get_nc():
    if "nc" not in _CACHE:
        _CACHE["nc"] = build_nc()
    return _CACHE["nc"]


def make_in_maps(inputs, adj, W, a1, a2):
    inputs = np.asarray(inputs, dtype=np.float32)
    adj = np.asarray(adj, dtype=np.float32)
    W = np.asarray(W, dtype=np.float32)
    a1 = np.asarray(a1, dtype=np.float32)
    a2 = np.asarray(a2, dtype=np.float32)

    # projections (~3% of FLOPs) on host, replicated to all cores
    Wh = inputs @ W
    f1 = (Wh @ a1).reshape(N).astype(np.float32)
    f2 = (Wh @ a2).reshape(N).astype(np.float32)
    whp = np.concatenate(
        [Wh, np.ones((N, 1), np.float32)], axis=1).astype(BF16_NP)
    # [128, CT*(D+1)]: row p holds [t, d] for c = t*128 + p
    whp_p = np.ascontiguousarray(
        whp.reshape(CT, 128, D + 1).transpose(1, 0, 2).reshape(128, -1))

    gp = np.exp(-(1.0 - ALPHA) * f1)          # per-row factor
    b1 = np.exp(f2)
    b2 = np.exp(ALPHA * f2)
    b1c = np.ascontiguousarray(b1.reshape(CT, 128).T)
    b2c = np.ascontiguousarray(b2.reshape(CT, 128).T)
    bv = np.ascontiguousarray(np.stack([b2c, b1c, -b1c], axis=1)
                              ).astype(np.float32)  # [128, 3, CT]

    in_maps = []
    for k in range(NCORES):
        r0, r1 = k * R, (k + 1) * R
        adjT = (adj[r0:r1, :].T > 0).astype(np.float32)  # [N, R] 0/1
        # split chunks: SW octo o = chunks 16o..16o+7, HW = 16o+8..16o+15
        a4 = adjT.reshape(NO, 16, 128, R)
        sw = np.ascontiguousarray(
            a4[:, :8].transpose(0, 2, 1, 3).reshape(NO * 128, 8 * R))
        hw = np.ascontiguousarray(
            a4[:, 8:].transpose(0, 2, 1, 3).reshape(NO * 128, 8 * R))
        in_maps.append({
            "adj8": sw.astype(FP8E4_NP),
            "adjb": hw.astype(BF16_NP),
            "whp": whp_p,
            "gbp": np.ascontiguousarray(np.broadcast_to(
                gp[r0:r1].reshape(1, R).astype(BF16_NP), (128, R))),
            "bv": bv,
        })
    return in_maps


def run(in_maps, trace=False):
    nc = _get_nc()
    res = bass_utils.run_bass_kernel_spmd(
        nc, [dict(m) for m in in_maps], core_ids=list(range(NCORES)),
        trace=trace,
    )
    out = np.concatenate([res.results[k]["out"] for k in range(NCORES)],
                         axis=0)
    return out, res


def kernel(inputs, adj, cmt_weight, W, a1, a2):
    in_maps = make_in_maps(inputs, adj, W, a1, a2)
    out, _ = run(in_maps, trace=False)
    return out.astype(np.float32)
